# revision 42
# baseline (speedup 1.0000x reference)
# Trainium2 Bass kernel for nn_ActorHead (masked multi-head cross-attention,
# returns (topic_align [N, dk], influence [N])).
#
# Sharding: 8 cores = 4 head-groups x 2 query-row shards. Core c = s*4 + g
# owns heads {2g, 2g+1} and query rows [2048*s, 2048*(s+1)).
# Each core computes sum_{h in group} softmax(QK^T/sqrt(dk) + weight, mask) @ V_h
# normalized per-row; host sums the 4 head-group partials, divides by H, and
# concatenates the two row shards. influence == 1.0 exactly (softmax rows sum
# to 1; reference takes mean over heads of row-sums).
#
# On-chip dataflow (per core), all matmuls bf16 with fp32 PSUM accumulation:
#   aT, bvT   : PE-transposed activations (bf16)
#   QT[h]     = WqT_h^T @ aT      [dk, Ns]
#   KT[h]     = WkT_h^T @ bvT     [dk, M]
#   V'[h]     = bv @ Wv_h^T       [M, dk] ++ ones column  [M, dk+1]
#   attT      = KT_mb^T @ QT      [m 128, n 512]  (transposed attention)
#   e0        = exp(attT / sqrt(dk))              (ACT, psum->sbuf bf16)
#   e         = e0 * ewT[mb]                      (DVE 2x bf16)
#     where ew = exp(weight) * mask  (ACT exp + GPSIMD mul, PE-transposed)
#   acc[h,j] += e[:, h, 128j:...]^T @ V'[h][mb]   [n 128, dk+1] (PSUM accum over m)
#   ta_j      = acc0/denom0 + acc1/denom1         (DVE reciprocal + stt)

import math

import numpy as np

import concourse.bass as bass
import concourse.tile as tile
from concourse import mybir
from concourse.masks import make_identity

F32 = mybir.dt.float32
BF16 = mybir.dt.bfloat16
U8 = mybir.dt.uint8

N, M, D, H = 4096, 4096, 1024, 8
DK = D // H            # 128
N_CORES = 8
HEADS_PER_CORE = 2
N_GROUPS = H // HEADS_PER_CORE      # 4 head groups
N_SHARDS = N_CORES // N_GROUPS      # 2 row shards
N_PER_CORE = N // N_SHARDS          # 2048


def split_multi_waits(nc, max_per_inst=1):
    """The walrus build in this environment rejects >1 sync-wait per
    instruction; split extras into single-wait NoOps on the same engine."""
    n_split = 0

    def process_block(b):
        nonlocal n_split
        insts = b.instructions
        out = []
        changed = False
        for inst in insts:
            si = inst.sync_info
            ow = list(si.on_wait) if si is not None else []
            if len(ow) > max_per_inst:
                head, tail = ow[:-max_per_inst], ow[-max_per_inst:]
                for k, w in enumerate(head):
                    nop = mybir.InstNoOp(name=f"{inst.name}-wsplit{k}", ins=[], outs=[])
                    nop.engine = inst.engine
                    nop.sync_info = mybir.SyncInfo(on_wait=[w], on_update=[])
                    out.append(nop)
                inst.sync_info = mybir.SyncInfo(on_wait=tail, on_update=list(si.on_update))
                changed = True
                n_split += len(head)
            out.append(inst)
        if changed:
            insts.clear()
            insts.extend(out)

    for fn in nc.m.functions:
        for b in fn.blocks:
            process_block(b)
            for sub in getattr(b, "blocks", []) or []:
                process_block(sub)
    return n_split


def build_program(n_rows=N_PER_CORE, m_total=M, d=D, heads=HEADS_PER_CORE,
                  dk=DK, n_tile=512):
    """Build the per-core SPMD Bass program. All cores run the same program on
    different input slices.

    Schedule: phase A merges the bv_z->KT/V' stream, ewT(0) production and the
    nt=0 attention m-loop into one pipeline (all engines busy from the start);
    phase B runs nt=1.. m-loops with double-buffered attention PSUM while
    producing ewT(nt+1) column-groups in the gaps.

    PSUM (8 banks): phase A: attA 1x2 + scrA 2 + acc 3 + ew-scratch 1.
                    phase B: attB 2x2 + acc 3 + ew-scratch 1.
    Accumulation chains share banks via per-element has_written: start=True
    only on the first matmul into a bank, stop=True only on the last.
    """
    assert n_rows % n_tile == 0 and n_tile % 128 == 0
    n_blk = n_rows // 128
    m_blk = m_total // 128
    d_chk = d // 128
    ntiles = n_rows // n_tile
    bpt = n_tile // 128
    scale = 1.0 / math.sqrt(dk)
    n_acc = heads * bpt

    nc = bass.Bass("TRN2", target_bir_lowering=False, debug=False)

    a_z = nc.dram_tensor("a_z", [n_rows, d], F32, kind="ExternalInput").ap()
    bv_z = nc.dram_tensor("bv_z", [m_total, d], F32, kind="ExternalInput").ap()
    mask = nc.dram_tensor("mask", [n_rows, m_total], U8, kind="ExternalInput").ap()
    weight = nc.dram_tensor("weight", [n_rows, m_total], F32, kind="ExternalInput").ap()
    wq = nc.dram_tensor("wq", [heads * dk, d], F32, kind="ExternalInput").ap()
    wk = nc.dram_tensor("wk", [heads * dk, d], F32, kind="ExternalInput").ap()
    wv = nc.dram_tensor("wv", [heads * dk, d], F32, kind="ExternalInput").ap()
    ta = nc.dram_tensor("ta", [n_rows, dk], F32, kind="ExternalOutput").ap()

    with tile.TileContext(nc) as tc:
        with tc.tile_pool(name="persist", bufs=1) as persist, \
             tc.tile_pool(name="ewt", bufs=2) as ewt_pool, \
             tc.tile_pool(name="wst", bufs=2) as wst, \
             tc.tile_pool(name="mst", bufs=2) as mst, \
             tc.tile_pool(name="est", bufs=2) as est, \
             tc.tile_pool(name="ewst", bufs=2) as ewst, \
             tc.tile_pool(name="epool", bufs=3) as epool, \
             tc.tile_pool(name="fin", bufs=4) as fin, \
             tc.tile_pool(name="acc_ps", bufs=1, space="PSUM") as acc_ps, \
             tc.tile_pool(name="ew_ps", bufs=1, space="PSUM") as ew_ps:

            ident = persist.tile([128, 128], BF16, tag="ident")
            make_identity(nc, ident)
            ident_f = persist.tile([128, 128], F32, tag="identf")
            make_identity(nc, ident_f)

            QT = persist.tile([128, heads, n_rows], BF16, tag="qt")
            KT = persist.tile([128, heads, m_total], BF16, tag="kt")
            VP = persist.tile([128, heads, m_blk, dk + 1], BF16, tag="vp")
            nc.vector.memset(VP[:, :, :, dk : dk + 1], 1.0)

            def new_ewT():
                return ewt_pool.tile([128, m_blk, n_tile], BF16, name="ewT", tag="ewT")

            # ewT column-group production: loads a [n_tile, 512] block of
            # weight/mask as [128, bpt, 512], exp+mask, transposes into 4 full
            # ewT m-columns so the m-loop can chase production.
            def produce_cg(nt, cg, ewT, mul_engine=None):
                rsl = slice(nt * n_tile, (nt + 1) * n_tile)
                csl = slice(512 * cg, 512 * (cg + 1))
                wcol = wst.tile([128, bpt, 512], F32, name="wcol", tag="wcol")
                nc.sync.dma_start(
                    out=wcol,
                    in_=weight[rsl, csl].rearrange("(b p) c -> p b c", p=128))
                mcol = mst.tile([128, bpt, 512], BF16, name="mcol", tag="mcol")
                nc.gpsimd.dma_start(
                    out=mcol,
                    in_=mask[rsl, csl].rearrange("(b p) c -> p b c", p=128))
                expw = est.tile([128, bpt, 512], BF16, name="expw", tag="expw")
                nc.scalar.activation(expw, wcol, mybir.ActivationFunctionType.Exp)
                ewc = ewst.tile([128, bpt, 512], BF16, name="ewc", tag="ewc")
                (mul_engine or nc.gpsimd).tensor_mul(ewc, expw, mcol)
                for i in range(4):
                    mb = 4 * cg + i
                    eps = ew_ps.tile([128, bpt, 128], BF16, name="ewps", tag="ewps")
                    for b in range(bpt):
                        nc.tensor.transpose(eps[:, b, :],
                                            ewc[:, b, 128 * i : 128 * (i + 1)], ident)
                    nc.vector.tensor_copy(out=ewT[:, mb, :], in_=eps)

            # attention m-loop pieces (psum pool passed per phase)
            def make_accs():
                accs = []
                for t in range((n_acc + 2) // 3):
                    w = min(3, n_acc - 3 * t)
                    accs.append(acc_ps.tile([128, w, dk + 1], F32,
                                            name=f"acc{t}", tag=f"acc{t}"))
                return accs

            def att_iter(nt, mb, ewT, accs, att_pool):
                nsl = slice(nt * n_tile, (nt + 1) * n_tile)
                att = att_pool.tile([128, heads, n_tile], F32, name="att", tag="att")
                for hh in range(heads):
                    nc.tensor.matmul(att[:, hh, :],
                                     KT[:, hh, 128 * mb : 128 * (mb + 1)],
                                     QT[:, hh, nsl], start=True, stop=True)
                e0 = epool.tile([128, heads, n_tile], BF16, name="e0", tag="e0")
                nc.scalar.activation(e0, att, mybir.ActivationFunctionType.Exp,
                                     scale=scale)
                e1 = epool.tile([128, heads, n_tile], BF16, name="e1", tag="e1")
                esl = ewT[:, mb, :]
                ew_b = bass.AP(tensor=esl.tensor, offset=esl.offset,
                               ap=[esl.ap[0], [0, heads], *esl.ap[1:]])
                nc.vector.tensor_mul(e1, e0, ew_b)
                for hh in range(heads):
                    for j in range(bpt):
                        idx = hh * bpt + j
                        ti, slot = divmod(idx, 3)
                        width = accs[ti].shape[1]
                        nc.tensor.matmul(accs[ti][:, slot, :],
                                         e1[:, hh, 128 * j : 128 * (j + 1)],
                                         VP[:, hh, mb, :],
                                         start=(mb == 0 and slot == 0),
                                         stop=(mb == m_blk - 1 and slot == width - 1))

            def finalize(nt, accs):
                def acc_slot(idx):
                    return accs[idx // 3][:, idx % 3, :]
                for j in range(bpt):
                    rds = []
                    for hh in range(heads):
                        rd = fin.tile([128, 1], F32, name=f"rd{hh}", tag=f"rd{hh}")
                        nc.vector.reciprocal(rd, acc_slot(hh * bpt + j)[:, dk : dk + 1])
                        rds.append(rd)
                    t0 = fin.tile([128, dk], F32, name="t0", tag="t0")
                    nc.vector.tensor_scalar_mul(t0, acc_slot(j)[:, :dk], rds[0])
                    for hh in range(1, heads):
                        t1 = fin.tile([128, dk], F32, name=f"t1_{hh}", tag=f"t1_{hh}")
                        nc.vector.scalar_tensor_tensor(
                            out=t1, in0=acc_slot(hh * bpt + j)[:, :dk],
                            scalar=rds[hh], in1=t0,
                            op0=mybir.AluOpType.mult, op1=mybir.AluOpType.add)
                        t0 = t1
                    row = nt * n_tile + 128 * j
                    nc.sync.dma_start(out=ta[row : row + 128, :], in_=t0)

            # ---------------- phase A: merged prep + nt=0 ----------------
            with tc.tile_pool(name="prep", bufs=1) as prep, \
                 tc.tile_pool(name="prep_st", bufs=4) as prep_st, \
                 tc.tile_pool(name="prep_tb", bufs=3) as prep_tb, \
                 tc.tile_pool(name="attA", bufs=1, space="PSUM") as attA, \
                 tc.tile_pool(name="scrA", bufs=2, space="PSUM") as scrA:

                wT = {}
                for name, src in (("q", wq), ("k", wk), ("v", wv)):
                    wxb = prep.tile([128, heads, d], BF16, name=f"w{name}b",
                                    tag="wxb", bufs=2)
                    for p in range(heads):
                        nc.gpsimd.dma_start(out=wxb[:, p, :],
                                            in_=src[128 * p : 128 * (p + 1), :])
                    wxT = prep.tile([128, d_chk, heads, 128], BF16, name=f"w{name}t",
                                    tag=f"w{name}t")
                    for dd in range(d_chk):
                        wps = scrA.tile([128, heads, 128], BF16, name="wps", tag="scr")
                        for p in range(heads):
                            nc.tensor.transpose(wps[:, p, :],
                                                wxb[:, p, 128 * dd : 128 * (dd + 1)],
                                                ident)
                        nc.vector.tensor_copy(out=wxT[:, dd], in_=wps)
                    wT[name] = wxT

                def stream_a_block(nb):
                    azb = prep_st.tile([128, d], BF16, name="azb", tag="azb")
                    nc.gpsimd.dma_start(out=azb, in_=a_z[128 * nb : 128 * (nb + 1), :])
                    tb = prep_tb.tile([128, d_chk, 128], BF16, name="aztb", tag="tb")
                    pst = scrA.tile([128, d_chk, 128], BF16, name="tps", tag="scr")
                    for dd in range(d_chk):
                        nc.tensor.transpose(pst[:, dd, :],
                                            azb[:, 128 * dd : 128 * (dd + 1)], ident)
                    nc.vector.tensor_copy(out=tb, in_=pst)
                    pq = scrA.tile([128, heads, 128], F32, name="pq", tag="scr")
                    for dd in range(d_chk):
                        for hh in range(heads):
                            nc.tensor.matmul(pq[:, hh, :], wT["q"][:, dd, hh],
                                             tb[:, dd, :],
                                             start=(dd == 0 and hh == 0),
                                             stop=(dd == d_chk - 1 and hh == heads - 1))
                    nc.vector.tensor_copy(
                        out=QT[:, :, 128 * nb : 128 * (nb + 1)], in_=pq)

                def stream_bv_block(mb):
                    bvb = prep_st.tile([128, d], BF16, name="bvb", tag="bvb")
                    nc.gpsimd.dma_start(out=bvb, in_=bv_z[128 * mb : 128 * (mb + 1), :])
                    tb = prep_tb.tile([128, d_chk, 128], BF16, name="bvtb", tag="tb")
                    pst = scrA.tile([128, d_chk, 128], BF16, name="tps2", tag="scr")
                    for dd in range(d_chk):
                        nc.tensor.transpose(pst[:, dd, :],
                                            bvb[:, 128 * dd : 128 * (dd + 1)], ident)
                    nc.vector.tensor_copy(out=tb, in_=pst)
                    pkv = scrA.tile([128, 2, heads, 128], F32, name="pkv", tag="scr")
                    for dd in range(d_chk):
                        for hh in range(heads):
                            nc.tensor.matmul(pkv[:, 0, hh, :], wT["k"][:, dd, hh],
                                             tb[:, dd, :],
                                             start=(dd == 0 and hh == 0), stop=False)
                        for hh in range(heads):
                            nc.tensor.matmul(pkv[:, 1, hh, :], tb[:, dd, :],
                                             wT["v"][:, dd, hh],
                                             start=False,
                                             stop=(dd == d_chk - 1 and hh == heads - 1))
                    nc.vector.tensor_copy(
                        out=KT[:, :, 128 * mb : 128 * (mb + 1)], in_=pkv[:, 0])
                    nc.vector.tensor_copy(out=VP[:, :, mb, :dk], in_=pkv[:, 1])

                for nb in range(bpt):
                    stream_a_block(nb)

                ew_tiles = {0: new_ewT()}
                if ntiles > 1:
                    ew_tiles[1] = new_ewT()
                accs0 = make_accs()
                next_a = bpt
                LAG = 4
                for mb in range(m_blk + LAG):
                    if mb < m_blk:
                        stream_bv_block(mb)
                        if mb % 4 == 3:
                            produce_cg(0, mb // 4, ew_tiles[0],
                                       mul_engine=nc.vector)
                        if mb % 4 == 1 and ntiles > 1:
                            produce_cg(1, mb // 4, ew_tiles[1],
                                       mul_engine=nc.vector)
                        if mb % 3 != 0 and next_a < n_blk:
                            stream_a_block(next_a)
                            next_a += 1
                    if mb >= LAG:
                        att_iter(0, mb - LAG, ew_tiles[0], accs0, attA)
                while next_a < n_blk:
                    stream_a_block(next_a)
                    next_a += 1

            # ---------------- phase B: nt = 1.. ----------------
            with tc.tile_pool(name="attB", bufs=2, space="PSUM") as attB:
                finalize(0, accs0)
                ew_tiles.pop(0)

                for nt in range(1, ntiles):
                    ewT = ew_tiles.pop(nt)
                    accs = make_accs()
                    if nt + 1 < ntiles:
                        ew_tiles[nt + 1] = new_ewT()
                    for mb in range(m_blk):
                        att_iter(nt, mb, ewT, accs, attB)
                        if nt + 1 < ntiles and mb % 4 == 3:
                            cg = mb // 4
                            produce_cg(nt + 1, cg, ew_tiles[nt + 1],
                                       mul_engine=(nc.vector if cg % 2 == 0
                                                   else nc.gpsimd))
                    finalize(nt, accs)

    return nc


# ---------------------------------------------------------------------------
# Host-side execution
# ---------------------------------------------------------------------------

_CACHE = {}


def _get_program():
    if "nc" not in _CACHE:
        _CACHE["nc"] = build_program()
    return _CACHE["nc"]


def _make_in_maps(a_z, bv_z, mask, weight, Wq, Wk, Wv):
    a_z = np.ascontiguousarray(np.asarray(a_z, dtype=np.float32))
    bv_z = np.ascontiguousarray(np.asarray(bv_z, dtype=np.float32))
    mask_u8 = np.ascontiguousarray(np.asarray(mask)).view(np.uint8)
    weight = np.ascontiguousarray(np.asarray(weight, dtype=np.float32))
    Wq = np.asarray(Wq, dtype=np.float32)
    Wk = np.asarray(Wk, dtype=np.float32)
    Wv = np.asarray(Wv, dtype=np.float32)

    in_maps = []
    for c in range(N_CORES):
        s, g = divmod(c, N_GROUPS)
        rows = slice(s * N_PER_CORE, (s + 1) * N_PER_CORE)
        hcols = slice(g * HEADS_PER_CORE * DK, (g + 1) * HEADS_PER_CORE * DK)
        in_maps.append({
            "a_z": a_z[rows],
            "bv_z": bv_z,
            "mask": mask_u8[rows],
            "weight": weight[rows],
            "wq": np.ascontiguousarray(Wq[hcols]),
            "wk": np.ascontiguousarray(Wk[hcols]),
            "wv": np.ascontiguousarray(Wv[hcols]),
        })
    return in_maps


def _get_runner():
    """Build (once) a cached jax-jitted SPMD executor for the program,
    mirroring concourse.bass2jax.run_bass_via_pjrt but reusing the jitted
    callable across calls."""
    if "runner" in _CACHE:
        return _CACHE["runner"]

    import jax
    from jax.experimental.shard_map import shard_map
    from jax.sharding import Mesh, PartitionSpec
    import concourse.bass2jax as b2j

    nc = _get_program()
    split_multi_waits(nc)
    b2j.install_neuronx_cc_hook()

    partition_name = (nc.partition_id_tensor.name
                      if nc.partition_id_tensor else None)
    in_names, out_names, out_avals, zero_outs = [], [], [], []
    for alloc in nc.m.functions[0].allocations:
        if not isinstance(alloc, mybir.MemoryLocationSet):
            continue
        name = alloc.memorylocations[0].name
        if alloc.kind == "ExternalInput":
            if name != partition_name:
                in_names.append(name)
        elif alloc.kind == "ExternalOutput":
            out_names.append(name)
            shape = tuple(alloc.tensor_shape)
            dtype = mybir.dt.np(alloc.dtype)
            out_avals.append(jax.core.ShapedArray(shape, dtype))
            zero_outs.append(np.zeros(shape, dtype))
    n_params = len(in_names)
    all_names = in_names + out_names
    if partition_name is not None:
        all_names = all_names + [partition_name]

    def _body(*args):
        operands = list(args)
        if partition_name is not None:
            operands.append(b2j.partition_id_tensor())
        outs = b2j._bass_exec_p.bind(
            *operands,
            out_avals=tuple(out_avals),
            in_names=tuple(all_names),
            out_names=tuple(out_names),
            lowering_input_output_aliases=(),
            sim_require_finite=True,
            sim_require_nnan=True,
            nc=nc,
        )
        return tuple(outs)

    devices = jax.devices()[:N_CORES]
    mesh = Mesh(np.asarray(devices), ("core",))
    n_outs = len(out_names)
    sharded = jax.jit(
        shard_map(_body, mesh=mesh,
                  in_specs=(PartitionSpec("core"),) * (n_params + n_outs),
                  out_specs=(PartitionSpec("core"),) * n_outs,
                  check_rep=False),
        donate_argnums=tuple(range(n_params, n_params + n_outs)),
        keep_unused=True,
    )

    _CACHE["sharded"] = sharded
    _CACHE["in_names"] = in_names
    _CACHE["zero_outs"] = zero_outs

    def run(in_maps, timing_reps=0):
        import time
        concat_in = [
            np.concatenate([np.asarray(in_maps[c][name]) for c in range(N_CORES)], axis=0)
            for name in in_names
        ]
        concat_zeros = [np.zeros((N_CORES * z.shape[0], *z.shape[1:]), z.dtype)
                        for z in zero_outs]
        out_arrs = sharded(*concat_in, *concat_zeros)
        jax.block_until_ready(out_arrs)
        dt_best = None
        if timing_reps:
            args = [jax.device_put(a) for a in concat_in]
            jax.block_until_ready(args)
            # pipelined: issue all dispatches, block once at the end, so the
            # axon round-trip latency overlaps across calls
            zz_all = [[np.zeros((N_CORES * z.shape[0], *z.shape[1:]), z.dtype)
                       for z in zero_outs] for _ in range(timing_reps)]
            outs = []
            t0 = time.perf_counter()
            for r in range(timing_reps):
                outs.append(sharded(*args, *zz_all[r]))
            jax.block_until_ready(outs)
            dt_best = (time.perf_counter() - t0) / timing_reps
        results = [
            {name: np.asarray(out_arrs[i]).reshape(N_CORES, *out_avals[i].shape)[c]
             for i, name in enumerate(out_names)}
            for c in range(N_CORES)
        ]
        return results, dt_best

    _CACHE["runner"] = run
    return run


def kernel(a_z, bv_z, mask, weight, Wq, Wk, Wv, h, _timing_reps=0):
    assert int(h) == H
    in_maps = _make_in_maps(a_z, bv_z, mask, weight, Wq, Wk, Wv)
    try:
        run = _get_runner()
        results, dt_best = run(in_maps, timing_reps=_timing_reps)
    except Exception:
        # transient device errors (e.g. NRT_EXEC_UNIT_UNRECOVERABLE) can
        # poison a dispatch; rebuild the jitted runner once and retry
        _CACHE.pop("runner", None)
        run = _get_runner()
        results, dt_best = run(in_maps, timing_reps=_timing_reps)

    shards = []
    for s in range(N_SHARDS):
        acc = np.zeros((N_PER_CORE, DK), np.float32)
        for g in range(N_GROUPS):
            acc += results[s * N_GROUPS + g]["ta"]
        shards.append(acc / np.float32(H))
    topic_align = np.concatenate(shards, axis=0)
    influence = np.ones(N, np.float32)
    if _timing_reps:
        kernel.last_time_s = dt_best
    return topic_align, influence


# revision 44
# speedup vs baseline: 1.0191x; 1.0191x over previous
# Trainium2 Bass kernel for nn_ActorHead (masked multi-head cross-attention,
# returns (topic_align [N, dk], influence [N])).
#
# Sharding: 8 cores = 4 head-groups x 2 query-row shards. Core c = s*4 + g
# owns heads {2g, 2g+1} and query rows [2048*s, 2048*(s+1)).
# Each core computes sum_{h in group} softmax(QK^T/sqrt(dk) + weight, mask) @ V_h
# normalized per-row; host sums the 4 head-group partials, divides by H, and
# concatenates the two row shards. influence == 1.0 exactly (softmax rows sum
# to 1; reference takes mean over heads of row-sums).
#
# On-chip dataflow (per core), all matmuls bf16 with fp32 PSUM accumulation:
#   aT, bvT   : PE-transposed activations (bf16)
#   QT[h]     = WqT_h^T @ aT      [dk, Ns]
#   KT[h]     = WkT_h^T @ bvT     [dk, M]
#   V'[h]     = bv @ Wv_h^T       [M, dk] ++ ones column  [M, dk+1]
#   attT      = KT_mb^T @ QT      [m 128, n 512]  (transposed attention)
#   e0        = exp(attT / sqrt(dk))              (ACT, psum->sbuf bf16)
#   e         = e0 * ewT[mb]                      (DVE 2x bf16)
#     where ew = exp(weight) * mask  (ACT exp + GPSIMD mul, PE-transposed)
#   acc[h,j] += e[:, h, 128j:...]^T @ V'[h][mb]   [n 128, dk+1] (PSUM accum over m)
#   ta_j      = acc0/denom0 + acc1/denom1         (DVE reciprocal + stt)

import math

import numpy as np

import concourse.bass as bass
import concourse.tile as tile
from concourse import mybir
from concourse.masks import make_identity

F32 = mybir.dt.float32
BF16 = mybir.dt.bfloat16
U8 = mybir.dt.uint8

N, M, D, H = 4096, 4096, 1024, 8
DK = D // H            # 128
N_CORES = 8
HEADS_PER_CORE = 2
N_GROUPS = H // HEADS_PER_CORE      # 4 head groups
N_SHARDS = N_CORES // N_GROUPS      # 2 row shards
N_PER_CORE = N // N_SHARDS          # 2048


def split_multi_waits(nc, max_per_inst=1):
    """The walrus build in this environment rejects >1 sync-wait per
    instruction; split extras into single-wait NoOps on the same engine."""
    n_split = 0

    def process_block(b):
        nonlocal n_split
        insts = b.instructions
        out = []
        changed = False
        for inst in insts:
            si = inst.sync_info
            ow = list(si.on_wait) if si is not None else []
            if len(ow) > max_per_inst:
                head, tail = ow[:-max_per_inst], ow[-max_per_inst:]
                for k, w in enumerate(head):
                    nop = mybir.InstNoOp(name=f"{inst.name}-wsplit{k}", ins=[], outs=[])
                    nop.engine = inst.engine
                    nop.sync_info = mybir.SyncInfo(on_wait=[w], on_update=[])
                    out.append(nop)
                inst.sync_info = mybir.SyncInfo(on_wait=tail, on_update=list(si.on_update))
                changed = True
                n_split += len(head)
            out.append(inst)
        if changed:
            insts.clear()
            insts.extend(out)

    for fn in nc.m.functions:
        for b in fn.blocks:
            process_block(b)
            for sub in getattr(b, "blocks", []) or []:
                process_block(sub)
    return n_split


def build_program(n_rows=N_PER_CORE, m_total=M, d=D, heads=HEADS_PER_CORE,
                  dk=DK, n_tile=512):
    """Build the per-core SPMD Bass program. All cores run the same program on
    different input slices.

    Schedule: phase A merges the bv_z->KT/V' stream, ewT(0) production and the
    nt=0 attention m-loop into one pipeline (all engines busy from the start);
    phase B runs nt=1.. m-loops with double-buffered attention PSUM while
    producing ewT(nt+1) column-groups in the gaps.

    PSUM (8 banks): phase A: attA 1x2 + scrA 2 + acc 3 + ew-scratch 1.
                    phase B: attB 2x2 + acc 3 + ew-scratch 1.
    Accumulation chains share banks via per-element has_written: start=True
    only on the first matmul into a bank, stop=True only on the last.
    """
    assert n_rows % n_tile == 0 and n_tile % 128 == 0
    n_blk = n_rows // 128
    m_blk = m_total // 128
    d_chk = d // 128
    ntiles = n_rows // n_tile
    bpt = n_tile // 128
    scale = 1.0 / math.sqrt(dk)
    n_acc = heads * bpt

    nc = bass.Bass("TRN2", target_bir_lowering=False, debug=False)

    a_z = nc.dram_tensor("a_z", [n_rows, d], F32, kind="ExternalInput").ap()
    bv_z = nc.dram_tensor("bv_z", [m_total, d], F32, kind="ExternalInput").ap()
    mask = nc.dram_tensor("mask", [n_rows, m_total], U8, kind="ExternalInput").ap()
    weight = nc.dram_tensor("weight", [n_rows, m_total], F32, kind="ExternalInput").ap()
    wq = nc.dram_tensor("wq", [heads * dk, d], F32, kind="ExternalInput").ap()
    wk = nc.dram_tensor("wk", [heads * dk, d], F32, kind="ExternalInput").ap()
    wv = nc.dram_tensor("wv", [heads * dk, d], F32, kind="ExternalInput").ap()
    ta = nc.dram_tensor("ta", [n_rows, dk], F32, kind="ExternalOutput").ap()

    with tile.TileContext(nc) as tc:
        with tc.tile_pool(name="persist", bufs=1) as persist, \
             tc.tile_pool(name="ewt", bufs=2) as ewt_pool, \
             tc.tile_pool(name="wst", bufs=2) as wst, \
             tc.tile_pool(name="mst", bufs=2) as mst, \
             tc.tile_pool(name="est", bufs=3) as est, \
             tc.tile_pool(name="ewst", bufs=2) as ewst, \
             tc.tile_pool(name="epool", bufs=3) as epool, \
             tc.tile_pool(name="fin", bufs=4) as fin, \
             tc.tile_pool(name="acc_ps", bufs=1, space="PSUM") as acc_ps, \
             tc.tile_pool(name="ew_ps", bufs=1, space="PSUM") as ew_ps:

            ident = persist.tile([128, 128], BF16, tag="ident")
            make_identity(nc, ident)
            ident_f = persist.tile([128, 128], F32, tag="identf")
            make_identity(nc, ident_f)

            QT = persist.tile([128, heads, n_rows], BF16, tag="qt")
            KT = persist.tile([128, heads, m_total], BF16, tag="kt")
            VP = persist.tile([128, heads, m_blk, dk + 1], BF16, tag="vp")
            nc.vector.memset(VP[:, :, :, dk : dk + 1], 1.0)

            def new_ewT():
                return ewt_pool.tile([128, m_blk, n_tile], BF16, name="ewT", tag="ewT")

            # ewT column-group production: loads a [n_tile, 512] block of
            # weight/mask as [128, bpt, 512], exp+mask, transposes into 4 full
            # ewT m-columns so the m-loop can chase production.
            def produce_cg(nt, cg, ewT, mul_engine=None):
                rsl = slice(nt * n_tile, (nt + 1) * n_tile)
                csl = slice(512 * cg, 512 * (cg + 1))
                wcol = wst.tile([128, bpt, 512], F32, name="wcol", tag="wcol")
                nc.sync.dma_start(
                    out=wcol,
                    in_=weight[rsl, csl].rearrange("(b p) c -> p b c", p=128))
                mcol = mst.tile([128, bpt, 512], BF16, name="mcol", tag="mcol")
                nc.gpsimd.dma_start(
                    out=mcol,
                    in_=mask[rsl, csl].rearrange("(b p) c -> p b c", p=128))
                expw = est.tile([128, bpt, 512], BF16, name="expw", tag="expw")
                nc.scalar.activation(expw, wcol, mybir.ActivationFunctionType.Exp)
                ewc = ewst.tile([128, bpt, 512], BF16, name="ewc", tag="ewc")
                (mul_engine or nc.gpsimd).tensor_mul(ewc, expw, mcol)
                for i in range(4):
                    mb = 4 * cg + i
                    eps = ew_ps.tile([128, bpt, 128], BF16, name="ewps", tag="ewps")
                    for b in range(bpt):
                        nc.tensor.transpose(eps[:, b, :],
                                            ewc[:, b, 128 * i : 128 * (i + 1)], ident)
                    nc.vector.tensor_copy(out=ewT[:, mb, :], in_=eps)

            # attention m-loop pieces (psum pool passed per phase)
            def make_accs():
                accs = []
                for t in range((n_acc + 2) // 3):
                    w = min(3, n_acc - 3 * t)
                    accs.append(acc_ps.tile([128, w, dk + 1], F32,
                                            name=f"acc{t}", tag=f"acc{t}"))
                return accs

            def att_iter(nt, mb, ewT, accs, att_pool):
                nsl = slice(nt * n_tile, (nt + 1) * n_tile)
                att = att_pool.tile([128, heads, n_tile], F32, name="att", tag="att")
                for hh in range(heads):
                    nc.tensor.matmul(att[:, hh, :],
                                     KT[:, hh, 128 * mb : 128 * (mb + 1)],
                                     QT[:, hh, nsl], start=True, stop=True)
                e0 = epool.tile([128, heads, n_tile], BF16, name="e0", tag="e0")
                nc.scalar.activation(e0, att, mybir.ActivationFunctionType.Exp,
                                     scale=scale)
                e1 = epool.tile([128, heads, n_tile], BF16, name="e1", tag="e1")
                esl = ewT[:, mb, :]
                ew_b = bass.AP(tensor=esl.tensor, offset=esl.offset,
                               ap=[esl.ap[0], [0, heads], *esl.ap[1:]])
                nc.vector.tensor_mul(e1, e0, ew_b)
                for hh in range(heads):
                    for j in range(bpt):
                        idx = hh * bpt + j
                        ti, slot = divmod(idx, 3)
                        width = accs[ti].shape[1]
                        nc.tensor.matmul(accs[ti][:, slot, :],
                                         e1[:, hh, 128 * j : 128 * (j + 1)],
                                         VP[:, hh, mb, :],
                                         start=(mb == 0 and slot == 0),
                                         stop=(mb == m_blk - 1 and slot == width - 1))

            def finalize(nt, accs):
                def acc_slot(idx):
                    return accs[idx // 3][:, idx % 3, :]
                for j in range(bpt):
                    rds = []
                    for hh in range(heads):
                        rd = fin.tile([128, 1], F32, name=f"rd{hh}", tag=f"rd{hh}")
                        nc.vector.reciprocal(rd, acc_slot(hh * bpt + j)[:, dk : dk + 1])
                        rds.append(rd)
                    t0 = fin.tile([128, dk], F32, name="t0", tag="t0")
                    nc.vector.tensor_scalar_mul(t0, acc_slot(j)[:, :dk], rds[0])
                    for hh in range(1, heads):
                        t1 = fin.tile([128, dk], F32, name=f"t1_{hh}", tag=f"t1_{hh}")
                        nc.vector.scalar_tensor_tensor(
                            out=t1, in0=acc_slot(hh * bpt + j)[:, :dk],
                            scalar=rds[hh], in1=t0,
                            op0=mybir.AluOpType.mult, op1=mybir.AluOpType.add)
                        t0 = t1
                    row = nt * n_tile + 128 * j
                    nc.sync.dma_start(out=ta[row : row + 128, :], in_=t0)

            # ---------------- phase A: merged prep + nt=0 ----------------
            with tc.tile_pool(name="prep", bufs=1) as prep, \
                 tc.tile_pool(name="prep_st", bufs=4) as prep_st, \
                 tc.tile_pool(name="prep_tb", bufs=3) as prep_tb, \
                 tc.tile_pool(name="attA", bufs=1, space="PSUM") as attA, \
                 tc.tile_pool(name="scrA", bufs=2, space="PSUM") as scrA:

                wT = {}
                for name, src in (("q", wq), ("k", wk), ("v", wv)):
                    wxb = prep.tile([128, heads, d], BF16, name=f"w{name}b",
                                    tag="wxb", bufs=2)
                    for p in range(heads):
                        nc.gpsimd.dma_start(out=wxb[:, p, :],
                                            in_=src[128 * p : 128 * (p + 1), :])
                    wxT = prep.tile([128, d_chk, heads, 128], BF16, name=f"w{name}t",
                                    tag=f"w{name}t")
                    for dd in range(d_chk):
                        wps = scrA.tile([128, heads, 128], BF16, name="wps", tag="scr")
                        for p in range(heads):
                            nc.tensor.transpose(wps[:, p, :],
                                                wxb[:, p, 128 * dd : 128 * (dd + 1)],
                                                ident)
                        nc.vector.tensor_copy(out=wxT[:, dd], in_=wps)
                    wT[name] = wxT

                def stream_a_block(nb):
                    azb = prep_st.tile([128, d], BF16, name="azb", tag="azb")
                    nc.gpsimd.dma_start(out=azb, in_=a_z[128 * nb : 128 * (nb + 1), :])
                    tb = prep_tb.tile([128, d_chk, 128], BF16, name="aztb", tag="tb")
                    pst = scrA.tile([128, d_chk, 128], BF16, name="tps", tag="scr")
                    for dd in range(d_chk):
                        nc.tensor.transpose(pst[:, dd, :],
                                            azb[:, 128 * dd : 128 * (dd + 1)], ident)
                    nc.vector.tensor_copy(out=tb, in_=pst)
                    pq = scrA.tile([128, heads, 128], F32, name="pq", tag="scr")
                    for dd in range(d_chk):
                        for hh in range(heads):
                            nc.tensor.matmul(pq[:, hh, :], wT["q"][:, dd, hh],
                                             tb[:, dd, :],
                                             start=(dd == 0 and hh == 0),
                                             stop=(dd == d_chk - 1 and hh == heads - 1))
                    nc.vector.tensor_copy(
                        out=QT[:, :, 128 * nb : 128 * (nb + 1)], in_=pq)

                def stream_bv_block(mb):
                    bvb = prep_st.tile([128, d], BF16, name="bvb", tag="bvb")
                    nc.gpsimd.dma_start(out=bvb, in_=bv_z[128 * mb : 128 * (mb + 1), :])
                    tb = prep_tb.tile([128, d_chk, 128], BF16, name="bvtb", tag="tb")
                    pst = scrA.tile([128, d_chk, 128], BF16, name="tps2", tag="scr")
                    for dd in range(d_chk):
                        nc.tensor.transpose(pst[:, dd, :],
                                            bvb[:, 128 * dd : 128 * (dd + 1)], ident)
                    nc.vector.tensor_copy(out=tb, in_=pst)
                    pkv = scrA.tile([128, 2, heads, 128], F32, name="pkv", tag="scr")
                    for dd in range(d_chk):
                        for hh in range(heads):
                            nc.tensor.matmul(pkv[:, 0, hh, :], wT["k"][:, dd, hh],
                                             tb[:, dd, :],
                                             start=(dd == 0 and hh == 0), stop=False)
                        for hh in range(heads):
                            nc.tensor.matmul(pkv[:, 1, hh, :], tb[:, dd, :],
                                             wT["v"][:, dd, hh],
                                             start=False,
                                             stop=(dd == d_chk - 1 and hh == heads - 1))
                    nc.vector.tensor_copy(
                        out=KT[:, :, 128 * mb : 128 * (mb + 1)], in_=pkv[:, 0])
                    nc.vector.tensor_copy(out=VP[:, :, mb, :dk], in_=pkv[:, 1])

                for nb in range(bpt):
                    stream_a_block(nb)

                ew_tiles = {0: new_ewT()}
                if ntiles > 1:
                    ew_tiles[1] = new_ewT()
                accs0 = make_accs()
                next_a = bpt
                LAG = 4
                for mb in range(m_blk + LAG):
                    if mb < m_blk:
                        stream_bv_block(mb)
                        if mb % 4 == 3:
                            produce_cg(0, mb // 4, ew_tiles[0],
                                       mul_engine=nc.vector)
                        if mb % 4 == 1 and ntiles > 1:
                            produce_cg(1, mb // 4, ew_tiles[1],
                                       mul_engine=nc.vector)
                        if mb % 3 != 0 and next_a < n_blk:
                            stream_a_block(next_a)
                            next_a += 1
                    if mb >= LAG:
                        att_iter(0, mb - LAG, ew_tiles[0], accs0, attA)
                while next_a < n_blk:
                    stream_a_block(next_a)
                    next_a += 1

            # ---------------- phase B: nt = 1.. ----------------
            with tc.tile_pool(name="attB", bufs=2, space="PSUM") as attB:
                finalize(0, accs0)
                ew_tiles.pop(0)

                for nt in range(1, ntiles):
                    ewT = ew_tiles.pop(nt)
                    accs = make_accs()
                    if nt + 1 < ntiles:
                        ew_tiles[nt + 1] = new_ewT()
                    for mb in range(m_blk):
                        att_iter(nt, mb, ewT, accs, attB)
                        if nt + 1 < ntiles and mb % 4 == 3:
                            cg = mb // 4
                            produce_cg(nt + 1, cg, ew_tiles[nt + 1],
                                       mul_engine=(nc.vector if cg % 2 == 0
                                                   else nc.gpsimd))
                    finalize(nt, accs)

    return nc


# ---------------------------------------------------------------------------
# Host-side execution
# ---------------------------------------------------------------------------

_CACHE = {}


def _get_program():
    if "nc" not in _CACHE:
        _CACHE["nc"] = build_program()
    return _CACHE["nc"]


def _make_in_maps(a_z, bv_z, mask, weight, Wq, Wk, Wv):
    a_z = np.ascontiguousarray(np.asarray(a_z, dtype=np.float32))
    bv_z = np.ascontiguousarray(np.asarray(bv_z, dtype=np.float32))
    mask_u8 = np.ascontiguousarray(np.asarray(mask)).view(np.uint8)
    weight = np.ascontiguousarray(np.asarray(weight, dtype=np.float32))
    Wq = np.asarray(Wq, dtype=np.float32)
    Wk = np.asarray(Wk, dtype=np.float32)
    Wv = np.asarray(Wv, dtype=np.float32)

    in_maps = []
    for c in range(N_CORES):
        s, g = divmod(c, N_GROUPS)
        rows = slice(s * N_PER_CORE, (s + 1) * N_PER_CORE)
        hcols = slice(g * HEADS_PER_CORE * DK, (g + 1) * HEADS_PER_CORE * DK)
        in_maps.append({
            "a_z": a_z[rows],
            "bv_z": bv_z,
            "mask": mask_u8[rows],
            "weight": weight[rows],
            "wq": np.ascontiguousarray(Wq[hcols]),
            "wk": np.ascontiguousarray(Wk[hcols]),
            "wv": np.ascontiguousarray(Wv[hcols]),
        })
    return in_maps


def _get_runner():
    """Build (once) a cached jax-jitted SPMD executor for the program,
    mirroring concourse.bass2jax.run_bass_via_pjrt but reusing the jitted
    callable across calls."""
    if "runner" in _CACHE:
        return _CACHE["runner"]

    import jax
    from jax.experimental.shard_map import shard_map
    from jax.sharding import Mesh, PartitionSpec
    import concourse.bass2jax as b2j

    nc = _get_program()
    split_multi_waits(nc)
    b2j.install_neuronx_cc_hook()

    partition_name = (nc.partition_id_tensor.name
                      if nc.partition_id_tensor else None)
    in_names, out_names, out_avals, zero_outs = [], [], [], []
    for alloc in nc.m.functions[0].allocations:
        if not isinstance(alloc, mybir.MemoryLocationSet):
            continue
        name = alloc.memorylocations[0].name
        if alloc.kind == "ExternalInput":
            if name != partition_name:
                in_names.append(name)
        elif alloc.kind == "ExternalOutput":
            out_names.append(name)
            shape = tuple(alloc.tensor_shape)
            dtype = mybir.dt.np(alloc.dtype)
            out_avals.append(jax.core.ShapedArray(shape, dtype))
            zero_outs.append(np.zeros(shape, dtype))
    n_params = len(in_names)
    all_names = in_names + out_names
    if partition_name is not None:
        all_names = all_names + [partition_name]

    def _body(*args):
        operands = list(args)
        if partition_name is not None:
            operands.append(b2j.partition_id_tensor())
        outs = b2j._bass_exec_p.bind(
            *operands,
            out_avals=tuple(out_avals),
            in_names=tuple(all_names),
            out_names=tuple(out_names),
            lowering_input_output_aliases=(),
            sim_require_finite=True,
            sim_require_nnan=True,
            nc=nc,
        )
        return tuple(outs)

    devices = jax.devices()[:N_CORES]
    mesh = Mesh(np.asarray(devices), ("core",))
    n_outs = len(out_names)
    sharded = jax.jit(
        shard_map(_body, mesh=mesh,
                  in_specs=(PartitionSpec("core"),) * (n_params + n_outs),
                  out_specs=(PartitionSpec("core"),) * n_outs,
                  check_rep=False),
        donate_argnums=tuple(range(n_params, n_params + n_outs)),
        keep_unused=True,
    )

    _CACHE["sharded"] = sharded
    _CACHE["in_names"] = in_names
    _CACHE["zero_outs"] = zero_outs

    def run(in_maps, timing_reps=0):
        import time
        concat_in = [
            np.concatenate([np.asarray(in_maps[c][name]) for c in range(N_CORES)], axis=0)
            for name in in_names
        ]
        concat_zeros = [np.zeros((N_CORES * z.shape[0], *z.shape[1:]), z.dtype)
                        for z in zero_outs]
        out_arrs = sharded(*concat_in, *concat_zeros)
        jax.block_until_ready(out_arrs)
        dt_best = None
        if timing_reps:
            args = [jax.device_put(a) for a in concat_in]
            jax.block_until_ready(args)
            # pipelined: issue all dispatches, block once at the end, so the
            # axon round-trip latency overlaps across calls
            zz_all = [[np.zeros((N_CORES * z.shape[0], *z.shape[1:]), z.dtype)
                       for z in zero_outs] for _ in range(timing_reps)]
            outs = []
            t0 = time.perf_counter()
            for r in range(timing_reps):
                outs.append(sharded(*args, *zz_all[r]))
            jax.block_until_ready(outs)
            dt_best = (time.perf_counter() - t0) / timing_reps
        results = [
            {name: np.asarray(out_arrs[i]).reshape(N_CORES, *out_avals[i].shape)[c]
             for i, name in enumerate(out_names)}
            for c in range(N_CORES)
        ]
        return results, dt_best

    _CACHE["runner"] = run
    return run


def kernel(a_z, bv_z, mask, weight, Wq, Wk, Wv, h, _timing_reps=0):
    assert int(h) == H
    in_maps = _make_in_maps(a_z, bv_z, mask, weight, Wq, Wk, Wv)
    try:
        run = _get_runner()
        results, dt_best = run(in_maps, timing_reps=_timing_reps)
    except Exception:
        # transient device errors (e.g. NRT_EXEC_UNIT_UNRECOVERABLE) can
        # poison a dispatch; rebuild the jitted runner once and retry
        _CACHE.pop("runner", None)
        run = _get_runner()
        results, dt_best = run(in_maps, timing_reps=_timing_reps)

    shards = []
    for s in range(N_SHARDS):
        acc = np.zeros((N_PER_CORE, DK), np.float32)
        for g in range(N_GROUPS):
            acc += results[s * N_GROUPS + g]["ta"]
        shards.append(acc / np.float32(H))
    topic_align = np.concatenate(shards, axis=0)
    influence = np.ones(N, np.float32)
    if _timing_reps:
        kernel.last_time_s = dt_best
    return topic_align, influence
